# revision 52
# baseline (speedup 1.0000x reference)
"""Trainium2 Bass kernel for a 2-step BasicNCA2D cell update.

Strategy (v2: fp8 DoubleRow)
----------------------------
Data-parallel over batch: 8 images, one per NeuronCore. Both NCA steps fused
on-chip. Per step:
    y  = depthwise_conv5x5(x, conv_w) + conv_b        (reflect padding)
    h  = relu([x, y] @ fc0_w + fc0_b)
    dx = h @ fc1_w
    x' = concat([x[..., :1], x[..., 1:] + dx])

conv+fc0 fused into accumulating matmuls (M[di,dj] = diag(conv_w)@fc0_w[24:]
(+fc0_w[:24] at center)), rows in groups of 4, channels padded 24->32 so
partitions hold (row, ch). x lives in per-stage fp8-e4m3 "mega tiles"
[128, 129, 516]: slot k = padded rows 4k..4k+3 (incl. 2-row/2-col reflect
halos). A group's 10 (block x shift) streams collapse to 5 fp8 DoubleRow
matmuls (lhsT [128,2,128] = A/B weights, rhs [:, g:g+2, dj:dj+512] spans two
adjacent slots) at 0.5 cycles/row. Weights are pre-scaled x64 into e4m3
range; relu commutes with the scale so h carries 64*h (bf16) and fc1 weights
absorb the 1/64.

The residual telescopes: out = x0 + dx0 + dx1. Both steps' fc1 matmuls
accumulate into one PSUM bank per group (start / stop split); stage-1's fp8
x1 tile is written directly as dxp+x0 (two staggered half-adds, quantize on
write), and the final residual adds PSUM + the original bf16 x0 (from
DRAM). gpsimd cannot read PSUM, so the final add is split: Act copies 304
PSUM cols to SBUF for a gpsimd add, DVE adds the rest. DMA is batched 4
slots/groups per transfer (HWDGE fixed cost), with DRAM layouts
[slot, row, ch, col] so the access patterns stay <=3 dims.

Per 4-row group-pass: PE 12 matmuls (10 fp8-DR + 2 bf16 fc1) ~1494ns, Act
2 relus + fres copy ~1660ns, DVE 2 x1-adds + fres rest ~1660ns, Pool halo
copies + fres add ~1100ns. Engine order is pinned with nosync dep edges
(the scheduler otherwise reorders into relu round-trip stalls); lags are
chosen so every consumer's producer is >=1 iteration old. Cost model:
~231us/core (baseline fp32r version: 667us).
"""

import os

import numpy as np
import ml_dtypes

import concourse.mybir as mybir
import concourse.tile as tile
from concourse import bacc
from concourse.bass_utils import run_bass_kernel_spmd

F32 = mybir.dt.float32
BF16 = mybir.dt.bfloat16
F8 = mybir.dt.float8e4
DR = mybir.MatmulPerfMode.DoubleRow

H = 512
W = 512
C = 24
CP = 32  # padded channels
HD = 32
NCORES = 8
NBLK = H // 4 + 1  # 129 slots per stage
NGRP = H // 4  # 128 output groups
SLOT = W + 4  # 516
S1 = 64.0  # fp8 weight pre-scale


def _build_nc(steps: int, repeat: int = 1):
    # exact subregion overlap checks: without this the dep tracker's work cap
    # makes disjoint mega-tile slot accesses look dependent (false sem waits)
    os.environ["TILE_EXHAUSTIVE_MEMORY_SHARE_CHECK"] = "1"
    nc = bacc.Bacc("TRN2", target_bir_lowering=False, debug=False)

    # [slot/group, row, channel, col] layouts: the (row, channel) partition
    # pair is contiguous so batched-DMA access patterns merge to <=3 dims
    X8 = nc.dram_tensor("X8", [NBLK, 4, CP, W + 4], F8, kind="ExternalInput")
    XH = nc.dram_tensor("XH", [NGRP, 4, CP, W], BF16, kind="ExternalInput")
    WAB8 = nc.dram_tensor("WAB8", [128, 5, 2, 128], F8, kind="ExternalInput")
    WCD = nc.dram_tensor("WCD", [128, 128], BF16, kind="ExternalInput")
    BIAS = nc.dram_tensor("BIAS", [128, 1], F32, kind="ExternalInput")
    Y = nc.dram_tensor("Y", [NGRP, 4, CP, W], BF16, kind="ExternalOutput")

    with tile.TileContext(nc) as tc:
        with (
            tc.tile_pool(name="wpool", bufs=1) as wpool,
            tc.tile_pool(name="xhpool", bufs=3) as xhpool,
            tc.tile_pool(name="hpool", bufs=4) as hpool,
            tc.tile_pool(name="opool", bufs=2) as opool,
            tc.tile_pool(name="tpool", bufs=3) as tpool,
            tc.tile_pool(name="pph", bufs=3, space="PSUM") as pph,
            tc.tile_pool(name="ppdx", bufs=5, space="PSUM") as ppdx,
        ):
            wab_t = wpool.tile([128, 5, 2, 128], F8, tag="wab")
            nc.sync.dma_start(wab_t[:], WAB8.ap())
            wc_t = wpool.tile([128, 128], BF16, tag="wc")
            nc.sync.dma_start(wc_t[:], WCD.ap())
            bias_t = wpool.tile([128, 1], F32, tag="bias")
            nc.sync.dma_start(bias_t[:], BIAS.ap())

            # per-stage fp8 mega tiles: slot k cols [k*SLOT, (k+1)*SLOT)
            megas = [wpool.tile([128, NBLK, SLOT], F8, tag=f"mega{s}",
                                name=f"mega{s}") for s in range(steps)]

            from concourse.tile import add_dep_helper

            last_in_chain = {}

            def chain(key, binst):
                """Pin per-engine instruction order with free (nosync) edges
                so the scheduler can't reorder within an engine."""
                prev = last_in_chain.get(key)
                if prev is not None:
                    add_dep_helper(binst.ins, prev.ins, sync=False,
                                   reason="emission-order pin")
                last_in_chain[key] = binst

            xh = {}  # group -> (tile, sub) bf16 x0 view [128, 512]
            o4 = {}  # group -> (tile, sub) out staging view
            hs = [dict() for _ in range(steps)]  # stage -> group -> h tile
            dxp = {}  # group -> PSUM accumulator tile

            def load_slots(k0):
                nk = min(4, NBLK - k0)
                chain("sp", nc.sync.dma_start(
                    megas[0][:, k0 : k0 + nk, :],
                    X8.ap()[k0 : k0 + nk, :, :, :].transpose([1, 2, 0, 3]),
                ))

            def load_xh4(g0):
                t = xhpool.tile([128, 4, 512], BF16, tag="xh", name=f"xh_{g0}")
                for j in range(4):
                    xh[g0 + j] = (t, j)
                chain("sp", nc.sync.dma_start(
                    t[:], XH.ap()[g0 : g0 + 4, :, :, :].transpose([1, 2, 0, 3])
                ))

            hps = [dict() for _ in range(2)]  # stage -> group -> conv psum

            def conv_mms(s, g, do_relu=True):
                """conv+fc0 DoubleRow matmuls (+ relu) for stage s, group g."""
                hp = pph.tile([128, 512], F32, tag="hp", name=f"hp{s}_{g}")
                for dj in range(5):
                    chain("pe", nc.tensor.matmul(
                        hp[:],
                        wab_t[:, dj, :, :],
                        megas[s][:, g : g + 2, dj : dj + 512],
                        start=(dj == 0),
                        stop=(dj == 4),
                        perf_mode=DR,
                    ))
                hps[s][g] = hp
                if do_relu:
                    relu(s, g)

            def relu(s, g):
                hp = hps[s].pop(g)
                h = hpool.tile([128, 512], BF16, tag="h", name=f"h{s}_{g}")
                chain("act", nc.scalar.activation(
                    h[:], hp[:], mybir.ActivationFunctionType.Relu,
                    bias=bias_t[:],
                ))
                hs[s][g] = h

            def fc1(s, g):
                last = s == steps - 1
                h = hs[s].pop(g)
                if s == 0:
                    d = ppdx.tile([128, 512], F32, tag="dx", name=f"dx_{g}")
                    dxp[g] = d
                else:
                    d = dxp[g]
                chain("pe", nc.tensor.matmul(
                    d[:], wc_t[:], h[:], start=(s == 0), stop=last
                ))

            def halo_cols(T, k):
                for vc, pc in ((0, 4), (1, 3), (514, 512), (515, 511)):
                    chain("pool", nc.gpsimd.tensor_copy(
                        T[:, k, vc : vc + 1], T[:, k, pc : pc + 1]
                    ))

            def x1_write1(g):
                """First half of fp8 x' = dx0 + x0 (slot g upper rows), plus
                slot g's halo cols: they need only x1w1(g) and last
                iteration's x1w2(g-1), which shortens the slot-ready chain
                feeding the s1 convs."""
                T = megas[1]
                xt, xj = xh[g]
                chain("dve", nc.vector.tensor_add(
                    T[64:128, g, 2:514], dxp[g][0:64, :], xt[0:64, xj, :]
                ))
                if g > 0:
                    halo_cols(T, g)

            def x1_write2(g):
                T = megas[1]
                xt, xj = xh[g]
                chain("dve", nc.vector.tensor_add(
                    T[0:64, g + 1, 2:514], dxp[g][64:128, :], xt[64:128, xj, :]
                ))
                if g == 0:
                    # top reflect rows: slot0 rows 0,1 = image rows 2,1
                    chain("pool", nc.gpsimd.tensor_copy(
                        T[0:32, 0, 2:514], T[0:32, 1, 2:514]
                    ))
                    chain("pool", nc.gpsimd.tensor_copy(
                        T[32:64, 0, 2:514], T[96:128, 0, 2:514]
                    ))
                    halo_cols(T, 0)
                if g == NGRP - 1:
                    # bottom reflect rows of slot 128: rows 514,515 = img 510,509
                    k = NBLK - 1
                    chain("pool", nc.gpsimd.tensor_copy(
                        T[64:96, k, 2:514], T[0:32, k, 2:514]
                    ))
                    chain("pool", nc.gpsimd.tensor_copy(
                        T[96:128, k, 2:514], T[96:128, k - 1, 2:514]
                    ))
                    halo_cols(T, k)

            FL = 304  # fres columns routed Act(copy)->Pool(add); rest on DVE

            def fres_store(g):
                """Final residual out = (dx0+dx1) + x0, then store (one DMA
                per 4 groups). gpsimd can't read PSUM, so the add is split:
                Act copies cols [0:FL] of the PSUM bank to SBUF and Pool
                (all-SBUF) adds them, DVE adds the rest straight from PSUM -
                balancing the three engines under the PE floor."""
                j = g % 4
                if j == 0:
                    o4[g // 4] = opool.tile(
                        [128, 4, 512], BF16, tag="out", name=f"out_{g // 4}"
                    )
                o = o4[g // 4]
                xt, xj = xh.pop(g)
                d = dxp.pop(g)
                tmp = tpool.tile([128, FL], BF16, tag="tmp", name=f"tmp_{g}")
                chain("act", nc.scalar.copy(tmp[:], d[:, 0:FL]))
                chain("pool", nc.gpsimd.tensor_add(
                    o[:, j, 0:FL], tmp[:], xt[:, xj, 0:FL]
                ))
                chain("dve", nc.vector.tensor_add(
                    o[:, j, FL:512], d[:, FL:512], xt[:, xj, FL:512]
                ))
                if j == 3 or g == NGRP - 1:
                    b = g // 4
                    chain("sp", nc.sync.dma_start(
                        Y.ap()[4 * b : 4 * b + 4, :, :, :].transpose([1, 2, 0, 3]),
                        o4.pop(b)[:],
                    ))

            lag_b0 = 1
            lag_a1 = 4
            lag_b1 = 5 if steps > 1 else lag_b0
            n_iters = NGRP + lag_b1 + 1
            for _rep in range(repeat):
                xh.clear()
                dxp.clear()
                for d_ in hs:
                    d_.clear()
                for m in range(n_iters):
                    if m == 0:
                        load_slots(0)
                        load_slots(4)
                    elif m % 4 == 0 and 4 * (m // 4 + 1) < NBLK:
                        load_slots(4 * (m // 4 + 1))
                    if m % 4 == 0 and m < NGRP:
                        load_xh4(m)
                    g0 = m if m < NGRP else None
                    g1 = m - lag_a1 if steps > 1 and 0 <= m - lag_a1 < NGRP else None
                    b0 = m - lag_b0 if 0 <= m - lag_b0 < NGRP else None
                    b1 = (
                        m - lag_b1
                        if steps > 1 and 0 <= m - lag_b1 < NGRP
                        else None
                    )
                    # Pinned PE order [s0-convs, s1-convs, fc1-s1, fc1-s0]
                    # balances the three serial loops: relu->fc1 round trips
                    # (both fc1s see >=1080ns slack), the dx-bank WAR (freed
                    # by the Act copy + DVE fres-rest, both first in their
                    # queues so the release lands early), and the x1w->halo->
                    # s1-conv slot chain (s1-convs at mid-iteration, 2
                    # iterations after the x1 writes). fres_store leads the
                    # Act/DVE/Pool queues: its deps are a full iteration old,
                    # so those engines start busy instead of idling while the
                    # first conv burst finishes.
                    # relu-s1 of the group whose convs ran last iteration
                    # leads the Act queue (its deps are an iteration old)
                    if b1 is not None:
                        relu(1, b1)
                    if g0 is not None:
                        conv_mms(0, g0)
                    if g1 is not None:
                        conv_mms(1, g1, do_relu=False)
                    if b1 is not None:
                        fc1(1, b1)
                    if b0 is not None:
                        fc1(0, b0)
                    # DVE queue [x1w1(b0), fres-rest(b1), x1w2(b0)]: the fres
                    # slice between the two x1 writes absorbs the PSUM-bank
                    # handoff latency between them
                    if b0 is not None and steps > 1:
                        x1_write1(b0)
                    if b1 is not None:
                        fres_store(b1)
                    if b0 is not None:
                        if steps == 1:
                            fres_store(b0)
                        else:
                            x1_write2(b0)

    nc.compile()
    return nc


_NC_CACHE = {}
_REPEAT = 1


def _get_nc(steps):
    key = (steps, _REPEAT)
    if key not in _NC_CACHE:
        _NC_CACHE[key] = _build_nc(steps, repeat=_REPEAT)
    return _NC_CACHE[key]


def _prep_weights(conv_w, conv_b, fc0_w, fc0_b, fc1_w):
    conv_w = np.asarray(conv_w, np.float64)[:, :, 0, :]  # [5,5,24]
    W1 = np.asarray(fc0_w, np.float64)[:C]  # [24,32]
    W2 = np.asarray(fc0_w, np.float64)[C:]  # [24,32]
    fc1_w = np.asarray(fc1_w, np.float64)  # [32,23]

    # M[ki, kj] = diag(conv_w[ki,kj]) @ W2 (+ W1 at center)
    M = conv_w[:, :, :, None] * W2[None, None, :, :]  # [5,5,24,32]
    M[2, 2] += W1

    WAB = np.zeros((2, 5, 128, 128), np.float64)
    for dj in range(5):
        for g in range(4):
            for f in range(4):
                ka = g - f  # di+2 for block A
                if g >= f and 0 <= ka <= 4:
                    WAB[0, dj, g * 32 : g * 32 + C, f * 32 : f * 32 + HD] = M[ka, dj]
                kb = g + 4 - f  # di+2 for block B
                if g <= f and 0 <= kb <= 4:
                    WAB[1, dj, g * 32 : g * 32 + C, f * 32 : f * 32 + HD] = M[kb, dj]

    # [128(k), 5(dj), 2(ab), 128(m)], pre-scaled by S1, e4m3
    WAB8 = np.ascontiguousarray(
        (WAB * S1).transpose(2, 1, 0, 3)
    ).astype(ml_dtypes.float8_e4m3)

    WC = np.zeros((128, 128), np.float64)
    for f in range(4):
        WC[f * 32 : f * 32 + HD, f * 32 + 1 : f * 32 + C] = fc1_w
    WCD = (WC / S1).astype(ml_dtypes.bfloat16)  # h carries S1*h

    bias_eff = (
        np.asarray(fc0_b, np.float64) + np.asarray(conv_b, np.float64) @ W2
    ) * S1
    BIAS = np.tile(bias_eff, 4).reshape(128, 1).astype(np.float32)
    return WAB8, WCD, BIAS


def _prep_x(x_chw):
    """x [B,C,H,W] fp32 -> (X8 [B,CP,516,516] e4m3, XH [B,CP,512,512] bf16)."""
    B = x_chw.shape[0]
    xp = np.zeros((B, CP, H + 4, W + 4), np.float32)
    xp[:, :C] = np.pad(x_chw, ((0, 0), (0, 0), (2, 2), (2, 2)), mode="reflect")
    # -> [B, slot, row, ch, col]
    X8 = np.ascontiguousarray(
        xp.reshape(B, CP, NBLK, 4, W + 4).transpose(0, 2, 3, 1, 4)
    ).astype(ml_dtypes.float8_e4m3)
    XHf = np.zeros((B, CP, H, W), np.float32)
    XHf[:, :C] = x_chw
    XHb = np.ascontiguousarray(
        XHf.reshape(B, CP, NGRP, 4, W).transpose(0, 2, 3, 1, 4)
    ).astype(ml_dtypes.bfloat16)
    return X8, XHb


def _run_pass(x_chw, WAB8, WCD, BIAS, steps):
    """One device invocation: `steps` NCA steps on x [B, C, H, W] fp32."""
    B = x_chw.shape[0]
    X8, XHb = _prep_x(x_chw)
    nc = _get_nc(steps)
    in_maps = [
        {"X8": X8[i % B], "XH": XHb[i % B], "WAB8": WAB8, "WCD": WCD, "BIAS": BIAS}
        for i in range(NCORES)
    ]
    res = run_bass_kernel_spmd(nc, in_maps, core_ids=list(range(NCORES)))
    globals()["LAST_RESULTS"] = res
    # Y [group, row, ch, col] -> [C, H, W]
    return np.stack(
        [
            res.results[i]["Y"]
            .transpose(2, 0, 1, 3)
            .reshape(CP, H, W)[:C]
            .astype(np.float32)
            for i in range(B)
        ]
    )


def kernel(x, conv_w, conv_b, fc0_w, fc0_b, fc1_w, steps):
    steps = int(steps)
    x = np.asarray(x, np.float32)
    B = x.shape[0]
    assert x.shape == (B, H, W, C) and 1 <= B <= NCORES, x.shape
    if steps <= 0:
        return x.copy()

    WAB8, WCD, BIAS = _prep_weights(conv_w, conv_b, fc0_w, fc0_b, fc1_w)
    x_chw = np.ascontiguousarray(x.transpose(0, 3, 1, 2))
    while steps > 0:
        n = 2 if steps >= 2 else 1
        x_chw = _run_pass(x_chw, WAB8, WCD, BIAS, n)
        steps -= n
    return np.ascontiguousarray(x_chw.transpose(0, 2, 3, 1)).astype(np.float32)


if __name__ == "__main__":
    rng = np.random.default_rng(0)
    inputs = {
        "x": rng.standard_normal((8, H, W, C), dtype=np.float32),
        "conv_w": (rng.standard_normal((5, 5, 1, C)) * 0.1).astype(np.float32),
        "conv_b": (rng.standard_normal((C,)) * 0.1).astype(np.float32),
        "fc0_w": (rng.standard_normal((2 * C, HD)) * 0.1).astype(np.float32),
        "fc0_b": (rng.standard_normal((HD,)) * 0.1).astype(np.float32),
        "fc1_w": (rng.standard_normal((HD, C - 1)) * 0.1).astype(np.float32),
        "steps": 2,
    }
    out = kernel(**inputs)
    print(out.shape, out.dtype)
